# revision 1
# baseline (speedup 1.0000x reference)
"""Trainium2 Bass kernel for nn_MultiHeadAttention (B=2, S=2048, E=1024, H=16, D=64).

Sharding: 8 NeuronCores, 2 heads per core (tensor parallel over heads).
Each core computes, for its 2 heads: q/k/v projections (transposed layout),
attention with softmax done in transposed-score space (sums via an extra
ones-column appended to V), and its partial of the output projection.
Host sums the 8 partials and adds the output bias.

All matmuls run as float32r (fp32 data, bf16-rate PE path, fp32 accumulate).
"""

import time

import numpy as np

import concourse.mybir as mybir
import concourse.tile as tile
from concourse import bacc
from concourse.bass_utils import run_bass_kernel_spmd
from concourse.masks import make_identity

F32 = mybir.dt.float32
F32R = mybir.dt.float32r
AF = mybir.ActivationFunctionType

# Problem shapes (hardcoded per contest contract)
B, S, E, H, D = 2, 2048, 1024, 16, 64
NCORES = 8
HPC = H // NCORES          # heads per core = 2
DH = HPC * D               # head dims per core = 128
P = 128                    # partitions
SC = 512                   # moving-dim chunk (fp32 max)
KO = E // P                # contraction tiles for projections = 8
NSC = B * S // SC          # s-chunks over both batches = 8
KI = S // P                # ki tiles per batch = 16
NQC = S // SC              # qi chunks per batch = 4
ST = B * S // P            # s-tiles over both batches = 32
VW = 2 * (D + 1)           # v_aug row width for 2 heads = 130


def build_kernel(tc, xt, wqkvt, bqkv, wot, partial):
    nc = tc.nc
    xt_r = xt.rearrange("(ko p) s -> p ko s", p=P)  # [128, 8, 4096]

    with (
        tc.tile_pool(name="persist", bufs=1) as persist,
        tc.tile_pool(name="stream", bufs=3) as stream,
        tc.tile_pool(name="small", bufs=3) as small,
    ):
        # ---- resident tensors ----
        # first xt chunk starts streaming before anything else, per E-tile
        # so the first matmul only waits for one 256KB slice
        xt0 = stream.tile([P, KO, SC], F32R, name="xt", tag="xt")
        xt1 = stream.tile([P, KO, SC], F32R, name="xt", tag="xt")
        wqkv_sb = persist.tile([P, KO, 3 * DH], F32R)   # [128, 8, 384]
        wqkvt_r = wqkvt.rearrange("(ko p) n -> p ko n", p=P)
        for ko in range(KO):
            nc.sync.dma_start(xt0[:, ko, :], xt_r[:, ko, 0:SC])
            nc.sync.dma_start(wqkv_sb[:, ko, :], wqkvt_r[:, ko, :])
        # chunk 1 also loads per E-tile slice so its matmuls can chase
        # slices instead of waiting for the whole 2MB transfer
        for ko in range(KO):
            nc.sync.dma_start(xt1[:, ko, :], xt_r[:, ko, SC:2 * SC])
        wot_sb = persist.tile([P, E], F32R)
        bias_sb = persist.tile([P, 3], F32)
        nc.sync.dma_start(bias_sb[:], bqkv.rearrange("(j p) o -> p (j o)", p=P))
        ident = persist.tile([P, P], F32)
        make_identity(nc, ident[:])

        qT_sb = persist.tile([P, B * S], F32R)   # [128, 4096] rows: 2 heads x 64 dims
        kT_sb = persist.tile([P, B * S], F32R)
        vT_sb = persist.tile([P, B * S], F32)
        v_sb = persist.tile([P, ST, VW], F32R)   # per s-tile: [vA(64) 1 vB(64) 1]
        attnT_sb = persist.tile([P, B * S], F32R)

        # ---- phase 1: q/k/v projections (transposed layout) + V transposes ----
        # v_sb per s-tile layout: [ones(1) vA(64) ones(1) vB(64)] so that the
        # softmax denominator (ones row) lands on PSUM partition 0 — the HW
        # partition_broadcast ucode reads physical partition 0 only.
        # ones columns via ACT: out = 0*x + 1 (DVE memset has no f32r encoding).
        for col in (0, D + 1):
            nc.scalar.activation(v_sb[:, :, col], ident[:, 0:ST],
                                 AF.Identity, bias=1.0, scale=0.0)
        with (
            tc.tile_pool(name="ps1", bufs=2, space="PSUM") as ps1,
            tc.tile_pool(name="pstr", bufs=3, space="PSUM") as pstr,
        ):
            for sc in range(NSC):
                cs = slice(sc * SC, (sc + 1) * SC)
                ps = [ps1.tile([P, SC], F32, name=f"proj{j}", tag=f"proj{j}",
                               bufs=(1 if j == 2 else 2)) for j in range(3)]
                if sc == 0:
                    xt_t = xt0
                elif sc == 1:
                    xt_t = xt1
                else:
                    xt_t = stream.tile([P, KO, SC], F32R, name="xt", tag="xt")
                    nc.sync.dma_start(xt_t[:], xt_r[:, :, cs])
                for ko in range(KO):
                    for j in range(3):
                        nc.tensor.matmul(
                            ps[j][:],
                            wqkv_sb[:, ko, j * DH:(j + 1) * DH],
                            xt_t[:, ko, :],
                            start=(ko == 0), stop=(ko == KO - 1),
                        )
                for j, dest in enumerate([qT_sb, kT_sb, vT_sb]):
                    nc.vector.tensor_scalar_add(dest[:, cs], ps[j][:],
                                                bias_sb[:, j:j + 1])
                # transpose this chunk's V tiles right away (fills DMA waits;
                # lets batch-0 attention start before batch-1 projections end)
                for t in range(sc * (SC // P), (sc + 1) * (SC // P)):
                    pt = pstr.tile([P, P], F32, tag="tr")
                    nc.tensor.transpose(pt[:], vT_sb[:, t * P:(t + 1) * P], ident[:])
                    # single fused copy into both heads' data columns
                    dst = v_sb[:, t, :].rearrange("p (h w) -> p h w", h=2)[:, :, 1:D + 1]
                    nc.vector.tensor_copy(dst, pt[:].rearrange("p (h d) -> p h d", h=2))

        # Wo loads late: it is first needed by the out-projection of the
        # first attention chunk, well after the startup DMA crunch.
        nc.sync.dma_start(wot_sb[:], wot)

        # ---- phase 2+3: attention and out-projection, interleaved per chunk ----
        # Scores for the two heads live at SBUF partitions 0-63 / 64-127, so
        # bass auto-assigns PE row tiles (0,0) / (64,0): adjacent score MMs
        # for head 0 and head 1 execute concurrently in the split PE array.
        with (
            tc.tile_pool(name="pssc", bufs=2, space="PSUM") as pssc,
            tc.tile_pool(name="psoa", bufs=1, space="PSUM") as psoa,
            tc.tile_pool(name="pspo", bufs=1, space="PSUM") as pspo,
        ):
            for b in range(B):
                for qc in range(NQC):
                    qs = slice(b * S + qc * SC, b * S + (qc + 1) * SC)
                    po = [psoa.tile([P, SC], F32, name=f"oa{h}", tag=f"oa{h}",
                                    bufs=(2 if h == 0 else 1))
                          for h in range(HPC)]
                    for ki in range(KI):
                        ks = b * S + ki * P
                        pss = pssc.tile([P, HPC, SC], F32, tag="sc")
                        for h in range(HPC):
                            ko64 = h * D
                            nc.tensor.matmul(
                                pss[:, h, :],
                                kT_sb[ko64:ko64 + D, ks:ks + P],
                                qT_sb[ko64:ko64 + D, qs],
                                start=True, stop=True,
                            )
                        # one exp over both heads' score tiles
                        ex = stream.tile([P, HPC, SC], F32R, tag="exp", bufs=4)
                        nc.scalar.activation(ex[:], pss[:], AF.Exp,
                                             scale=1.0 / np.sqrt(D))
                        for h in range(HPC):
                            nc.tensor.matmul(
                                po[h][0:D + 1, :],
                                v_sb[:, b * KI + ki, h * (D + 1):(h + 1) * (D + 1)],
                                ex[:, h, :],
                                start=(ki == 0), stop=(ki == KI - 1),
                                skip_group_check=True,
                            )
                    for h in range(HPC):
                        ko64 = h * D
                        # evacuate accumulator first so its PSUM bank frees
                        # quickly; normalize from SBUF afterwards.
                        oa = small.tile([D + 1, SC], F32, tag="oa_sb")
                        nc.vector.tensor_copy(oa[:], po[h][0:D + 1, :])
                        # normalize: attnT = unnorm / Z  (Z in row 0)
                        recip = small.tile([1, SC], F32, tag="recip")
                        nc.vector.reciprocal(recip[:], oa[0:1, :])
                        bc = small.tile([D + 1, SC], F32, tag="bc")
                        nc.gpsimd.partition_broadcast(bc[:], recip[:])
                        nrm = small.tile([D + 1, SC], F32R, tag="nrm")
                        nc.vector.tensor_mul(nrm[:], oa[:], bc[:])
                        nc.sync.dma_start(attnT_sb[ko64:ko64 + D, qs], nrm[1:D + 1, :])

                    # out-projection for this chunk's 4 s-tiles (both heads done)
                    for sti in range(SC // P):
                        row = b * S + qc * SC + sti * P
                        ot = stream.tile([P, E], F32, tag="ot")
                        for ne in range(E // SC):
                            pp = pspo.tile([P, SC], F32, tag="po")
                            nc.tensor.matmul(
                                pp[:],
                                attnT_sb[:, row:row + P],
                                wot_sb[:, ne * SC:(ne + 1) * SC],
                                start=True, stop=True,
                            )
                            nc.vector.tensor_copy(ot[:, ne * SC:(ne + 1) * SC], pp[:])
                        nc.sync.dma_start(partial[row:row + P, :], ot[:])


def build_module():
    nc = bacc.Bacc("TRN2", target_bir_lowering=False, debug=False,
                   num_devices=NCORES)
    xt = nc.dram_tensor("xt", [E, B * S], F32R, kind="ExternalInput").ap()
    wqkvt = nc.dram_tensor("wqkvt", [E, 3 * DH], F32R, kind="ExternalInput").ap()
    bqkv = nc.dram_tensor("bqkv", [3 * DH, 1], F32, kind="ExternalInput").ap()
    wot = nc.dram_tensor("wot", [DH, E], F32R, kind="ExternalInput").ap()
    partial = nc.dram_tensor("partial", [B * S, E], F32, kind="ExternalOutput").ap()
    with tile.TileContext(nc) as tc:
        build_kernel(tc, xt, wqkvt, bqkv, wot, partial)
    nc.compile()
    return nc


def make_in_maps(x, Wq, bq, Wk, bk, Wv, bv, Wo, bo):
    xt = np.ascontiguousarray(x.reshape(B * S, E).T).astype(np.float32)
    in_maps = []
    for c in range(NCORES):
        rows = slice(c * DH, (c + 1) * DH)
        wqkvt = np.ascontiguousarray(
            np.concatenate([Wq[rows], Wk[rows], Wv[rows]], axis=0).T
        ).astype(np.float32)
        bqkv = np.concatenate([bq[rows], bk[rows], bv[rows]]).reshape(3 * DH, 1)
        wot = np.ascontiguousarray(Wo[:, rows].T).astype(np.float32)
        in_maps.append({
            "xt": xt,
            "wqkvt": wqkvt,
            "bqkv": bqkv.astype(np.float32),
            "wot": wot,
        })
    return in_maps


_NC_CACHE = None


def kernel(x, Wq, bq, Wk, bk, Wv, bv, Wo, bo, _trace=False):
    global _NC_CACHE
    x = np.asarray(x)
    if _NC_CACHE is None:
        _NC_CACHE = build_module()
    nc = _NC_CACHE
    in_maps = make_in_maps(np.asarray(x), np.asarray(Wq), np.asarray(bq),
                           np.asarray(Wk), np.asarray(bk), np.asarray(Wv),
                           np.asarray(bv), np.asarray(Wo), np.asarray(bo))
    # transient NRT_EXEC_UNIT_UNRECOVERABLE flakes have been observed on this
    # fabric; a short-delay retry has always succeeded.
    last_err = None
    for attempt in range(3):
        try:
            res = run_bass_kernel_spmd(nc, in_maps, core_ids=list(range(NCORES)),
                                       trace=_trace)
            break
        except Exception as e:  # noqa: BLE001
            last_err = e
            time.sleep(10 * (attempt + 1))
    else:
        raise last_err
    partials = np.stack([res.results[c]["partial"] for c in range(NCORES)])
    out = partials.sum(axis=0, dtype=np.float64) + np.asarray(bo, dtype=np.float64)
    out = out.astype(np.float32).reshape(B, S, E)
    if _trace:
        return out, res
    return out



# revision 24
# speedup vs baseline: 1.0238x; 1.0238x over previous
"""Trainium2 Bass kernel for nn_MultiHeadAttention (B=2, S=2048, E=1024, H=16, D=64).

Sharding: 8 NeuronCores, 2 heads per core (tensor parallel over heads).
Each core computes, for its 2 heads: q/k/v projections (transposed layout),
attention with softmax done in transposed-score space (sums via an extra
ones-column appended to V), and its partial of the output projection.
Host sums the 8 partials and adds the output bias.

All matmuls run as float32r (fp32 data, bf16-rate PE path, fp32 accumulate).
"""

import time

import numpy as np

import concourse.mybir as mybir
import concourse.tile as tile
from concourse import bacc
from concourse.bass_utils import run_bass_kernel_spmd
from concourse.masks import make_identity

F32 = mybir.dt.float32
F32R = mybir.dt.float32r
AF = mybir.ActivationFunctionType

# Problem shapes (hardcoded per contest contract)
B, S, E, H, D = 2, 2048, 1024, 16, 64
NCORES = 8
HPC = H // NCORES          # heads per core = 2
DH = HPC * D               # head dims per core = 128
P = 128                    # partitions
SC = 512                   # moving-dim chunk (fp32 max)
KO = E // P                # contraction tiles for projections = 8
NSC = B * S // SC          # s-chunks over both batches = 8
KI = S // P                # ki tiles per batch = 16
NQC = S // SC              # qi chunks per batch = 4
ST = B * S // P            # s-tiles over both batches = 32
VW = 2 * (D + 1)           # v_aug row width for 2 heads = 130


def build_kernel(tc, xt, wqkvt, bqkv, wot, partial):
    nc = tc.nc
    xt_r = xt.rearrange("(ko p) s -> p ko s", p=P)  # [128, 8, 4096]

    with (
        tc.tile_pool(name="persist", bufs=1) as persist,
        tc.tile_pool(name="stream", bufs=3) as stream,
        tc.tile_pool(name="small", bufs=3) as small,
    ):
        # ---- resident tensors ----
        # first xt chunk starts streaming before anything else, per E-tile
        # so the first matmul only waits for one 256KB slice
        xt0 = stream.tile([P, KO, SC], F32R, name="xt", tag="xt")
        xt1 = stream.tile([P, KO, SC], F32R, name="xt", tag="xt")
        wqkv_sb = persist.tile([P, KO, 3 * DH], F32R)   # [128, 8, 384]
        wqkvt_r = wqkvt.rearrange("(ko p) n -> p ko n", p=P)
        for ko in range(KO):
            nc.sync.dma_start(xt0[:, ko, :], xt_r[:, ko, 0:SC])
            nc.sync.dma_start(wqkv_sb[:, ko, :], wqkvt_r[:, ko, :])
        # chunk 1 also loads per E-tile slice so its matmuls can chase
        # slices instead of waiting for the whole 2MB transfer
        for ko in range(KO):
            nc.sync.dma_start(xt1[:, ko, :], xt_r[:, ko, SC:2 * SC])
        wot_sb = persist.tile([P, E], F32R)
        bias_sb = persist.tile([P, 3], F32)
        nc.sync.dma_start(bias_sb[:], bqkv.rearrange("(j p) o -> p (j o)", p=P))
        ident = persist.tile([P, P], F32)
        make_identity(nc, ident[:])

        qT_sb = persist.tile([P, B * S], F32R)   # [128, 4096] rows: 2 heads x 64 dims
        kT_sb = persist.tile([P, B * S], F32R)
        vT_sb = persist.tile([P, B * S], F32)
        v_sb = persist.tile([P, ST, VW], F32R)   # per s-tile: [vA(64) 1 vB(64) 1]
        attnT_sb = persist.tile([P, B * S], F32R)

        # ---- phase 1: q/k/v projections (transposed layout) + V transposes ----
        # v_sb per s-tile layout: [ones(1) vA(64) ones(1) vB(64)] so that the
        # softmax denominator (ones row) lands on PSUM partition 0 — the HW
        # partition_broadcast ucode reads physical partition 0 only.
        # ones columns via ACT: out = 0*x + 1 (DVE memset has no f32r encoding).
        for col in (0, D + 1):
            nc.scalar.activation(v_sb[:, :, col], ident[:, 0:ST],
                                 AF.Identity, bias=1.0, scale=0.0)
        with (
            tc.tile_pool(name="ps1", bufs=2, space="PSUM") as ps1,
            tc.tile_pool(name="pstr", bufs=3, space="PSUM") as pstr,
        ):
            for sc in range(NSC):
                cs = slice(sc * SC, (sc + 1) * SC)
                ps = [ps1.tile([P, SC], F32, name=f"proj{j}", tag=f"proj{j}",
                               bufs=(1 if j == 2 else 2)) for j in range(3)]
                if sc == 0:
                    xt_t = xt0
                elif sc == 1:
                    xt_t = xt1
                else:
                    xt_t = stream.tile([P, KO, SC], F32R, name="xt", tag="xt")
                    nc.sync.dma_start(xt_t[:], xt_r[:, :, cs])
                for ko in range(KO):
                    for j in range(3):
                        nc.tensor.matmul(
                            ps[j][:],
                            wqkv_sb[:, ko, j * DH:(j + 1) * DH],
                            xt_t[:, ko, :],
                            start=(ko == 0), stop=(ko == KO - 1),
                        )
                for j, dest in enumerate([qT_sb, kT_sb, vT_sb]):
                    nc.vector.tensor_scalar_add(dest[:, cs], ps[j][:],
                                                bias_sb[:, j:j + 1])
                # transpose this chunk's V tiles right away (fills DMA waits;
                # lets batch-0 attention start before batch-1 projections end)
                for t in range(sc * (SC // P), (sc + 1) * (SC // P)):
                    pt = pstr.tile([P, P], F32, tag="tr")
                    nc.tensor.transpose(pt[:], vT_sb[:, t * P:(t + 1) * P], ident[:])
                    # single fused copy into both heads' data columns
                    dst = v_sb[:, t, :].rearrange("p (h w) -> p h w", h=2)[:, :, 1:D + 1]
                    nc.vector.tensor_copy(dst, pt[:].rearrange("p (h d) -> p h d", h=2))

        # Wo loads late: it is first needed by the out-projection of the
        # first attention chunk, well after the startup DMA crunch.
        nc.sync.dma_start(wot_sb[:], wot)

        # ---- phase 2+3: attention and out-projection, interleaved per chunk ----
        # Scores for the two heads live at SBUF partitions 0-63 / 64-127, so
        # bass auto-assigns PE row tiles (0,0) / (64,0): adjacent score MMs
        # for head 0 and head 1 execute concurrently in the split PE array.
        with (
            tc.tile_pool(name="pssc", bufs=2, space="PSUM") as pssc,
            tc.tile_pool(name="psoa", bufs=1, space="PSUM") as psoa,
            tc.tile_pool(name="pspo", bufs=1, space="PSUM") as pspo,
        ):
            for b in range(B):
                for qc in range(NQC):
                    qs = slice(b * S + qc * SC, b * S + (qc + 1) * SC)
                    po = [psoa.tile([P, SC], F32, name=f"oa{h}", tag=f"oa{h}",
                                    bufs=(2 if h == 0 else 1))
                          for h in range(HPC)]
                    for ki in range(KI):
                        ks = b * S + ki * P
                        pss = pssc.tile([P, HPC, SC], F32, tag="sc")
                        for h in range(HPC):
                            ko64 = h * D
                            nc.tensor.matmul(
                                pss[:, h, :],
                                kT_sb[ko64:ko64 + D, ks:ks + P],
                                qT_sb[ko64:ko64 + D, qs],
                                start=True, stop=True,
                            )
                        # one exp over both heads' score tiles
                        ex = stream.tile([P, HPC, SC], F32R, tag="exp", bufs=4)
                        nc.scalar.activation(ex[:], pss[:], AF.Exp,
                                             scale=1.0 / np.sqrt(D))
                        for h in range(HPC):
                            nc.tensor.matmul(
                                po[h][0:D + 1, :],
                                v_sb[:, b * KI + ki, h * (D + 1):(h + 1) * (D + 1)],
                                ex[:, h, :],
                                start=(ki == 0), stop=(ki == KI - 1),
                                skip_group_check=True,
                            )
                    for h in range(HPC):
                        ko64 = h * D
                        # evacuate accumulator first so its PSUM bank frees
                        # quickly; normalize from SBUF afterwards.
                        oa = small.tile([D + 1, SC], F32, tag="oa_sb")
                        nc.vector.tensor_copy(oa[:], po[h][0:D + 1, :])
                        # normalize: attnT = unnorm / Z  (Z in row 0)
                        recip = small.tile([1, SC], F32, tag="recip")
                        nc.vector.reciprocal(recip[:], oa[0:1, :])
                        bc = small.tile([D + 1, SC], F32, tag="bc")
                        nc.gpsimd.partition_broadcast(bc[:], recip[:])
                        nrm = small.tile([D + 1, SC], F32R, tag="nrm")
                        nc.vector.tensor_mul(nrm[:], oa[:], bc[:])
                        nc.sync.dma_start(attnT_sb[ko64:ko64 + D, qs], nrm[1:D + 1, :])

                    # out-projection for this chunk's 4 s-tiles (both heads done)
                    for sti in range(SC // P):
                        row = b * S + qc * SC + sti * P
                        ot = stream.tile([P, E], F32, tag="ot")
                        for ne in range(E // SC):
                            pp = pspo.tile([P, SC], F32, tag="po")
                            nc.tensor.matmul(
                                pp[:],
                                attnT_sb[:, row:row + P],
                                wot_sb[:, ne * SC:(ne + 1) * SC],
                                start=True, stop=True,
                            )
                            nc.vector.tensor_copy(ot[:, ne * SC:(ne + 1) * SC], pp[:])
                        nc.sync.dma_start(partial[row:row + P, :], ot[:])


def build_module():
    nc = bacc.Bacc("TRN2", target_bir_lowering=False, debug=False,
                   num_devices=NCORES)
    xt = nc.dram_tensor("xt", [E, B * S], F32R, kind="ExternalInput").ap()
    wqkvt = nc.dram_tensor("wqkvt", [E, 3 * DH], F32R, kind="ExternalInput").ap()
    bqkv = nc.dram_tensor("bqkv", [3 * DH, 1], F32, kind="ExternalInput").ap()
    wot = nc.dram_tensor("wot", [DH, E], F32R, kind="ExternalInput").ap()
    partial = nc.dram_tensor("partial", [B * S, E], F32, kind="ExternalOutput").ap()
    with tile.TileContext(nc) as tc:
        build_kernel(tc, xt, wqkvt, bqkv, wot, partial)
    nc.compile()
    return nc


def make_in_maps(x, Wq, bq, Wk, bk, Wv, bv, Wo, bo):
    xt = np.ascontiguousarray(x.reshape(B * S, E).T).astype(np.float32)
    in_maps = []
    for c in range(NCORES):
        rows = slice(c * DH, (c + 1) * DH)
        wqkvt = np.ascontiguousarray(
            np.concatenate([Wq[rows], Wk[rows], Wv[rows]], axis=0).T
        ).astype(np.float32)
        bqkv = np.concatenate([bq[rows], bk[rows], bv[rows]]).reshape(3 * DH, 1)
        wot = np.ascontiguousarray(Wo[:, rows].T).astype(np.float32)
        in_maps.append({
            "xt": xt,
            "wqkvt": wqkvt,
            "bqkv": bqkv.astype(np.float32),
            "wot": wot,
        })
    return in_maps


_NC_CACHE = None


def kernel(x, Wq, bq, Wk, bk, Wv, bv, Wo, bo, _trace=False):
    global _NC_CACHE
    x = np.asarray(x)
    if _NC_CACHE is None:
        _NC_CACHE = build_module()
    nc = _NC_CACHE
    in_maps = make_in_maps(np.asarray(x), np.asarray(Wq), np.asarray(bq),
                           np.asarray(Wk), np.asarray(bk), np.asarray(Wv),
                           np.asarray(bv), np.asarray(Wo), np.asarray(bo))
    # transient NRT_EXEC_UNIT_UNRECOVERABLE flakes have been observed on this
    # fabric; a short-delay retry has always succeeded.
    last_err = None
    for attempt in range(3):
        try:
            res = run_bass_kernel_spmd(nc, in_maps, core_ids=list(range(NCORES)),
                                       trace=_trace)
            break
        except Exception as e:  # noqa: BLE001
            last_err = e
            time.sleep(10 * (attempt + 1))
    else:
        raise last_err
    partials = np.stack([res.results[c]["partial"] for c in range(NCORES)])
    out = partials.sum(axis=0, dtype=np.float64) + np.asarray(bo, dtype=np.float64)
    out = out.astype(np.float32).reshape(B, S, E)
    if _trace:
        return out, res
    return out

